# revision 5
# baseline (speedup 1.0000x reference)
"""CTPN loss kernel for 8 Trainium2 NeuronCores.

Strategy (data parallel over anchors, maps sharded by position):
  * The H*W=24576 spatial positions are split into 8 contiguous slices of
    3072; core c holds the dense map data for its slice, re-laid-out into an
    SBUF-friendly [128, 1536] f32 tile of "channel-half" rows.
  * All index lists (positive/negative/vertical/side) are bucketed on the
    host by position -> core, and inside a core by (channel, pos-half) ->
    16-partition GPSIMD group.  One InstIndirectCopy per core gathers every
    referenced value (the gather is the whole memory-bound core of this
    loss).
  * Smooth-L1 is evaluated with the identity
        sl1(d) = 0.5*m^2 + |d| - m,   m = min(|d|, 1)
    so only three masked free-dim reductions are needed; per-partition
    partial sums go back to the host, which applies the per-segment
    divisors (1/(2*Nv), 1/No, 1/Ns) and sums across cores (the all-reduce).
  * Classification CE uses ce_pos = softplus(l0-l1), ce_neg = softplus(l1-l0)
    on pair-adjacent gathered columns.
"""

import sys

sys.path.insert(0, "/opt/trn_rl_repo")

import numpy as np

import concourse.bacc as bacc
import concourse.tile as tile
from concourse import mybir
from concourse import bass_utils

# ---------------- problem constants (hardcoded per contract) ----------------
H, W, K = 128, 192, 10
HW = H * W                     # 24576
N_CORES = 8
PPC = HW // N_CORES            # 3072 positions per core
COLS = 1536                    # slot width (elements) = half of PPC
QCOLS = 768                    # quarter width (score slots are pair-interleaved)
NS = 128.0
NV_REG = 20000
NO_REG = 5000

# ---- static unit tables ----------------------------------------------------
# unit kinds: 'vp' (a, h) -> 2 partitions; 'sd' (a, h) -> 1; 'sc' (a, q) -> 1
UNITS = []
for a in range(K):
    for h in range(2):
        UNITS.append(("vp", a, h))
for a in range(K):
    for h in range(2):
        UNITS.append(("sd", a, h))
for a in range(K):
    for q in range(4):
        UNITS.append(("sc", a, q))
N_UNITS = len(UNITS)  # 80
UNIT_NPART = {"vp": 2, "sd": 1, "sc": 1}

_cache = {}


def _pack_units(main_cnt, cls_cnt):
    """Greedy LPT bin-pack of units into 8 groups of <=16 partitions.

    main_cnt/cls_cnt: [N_UNITS] entry counts for one core.
    Returns: group id per unit, per-group (n_main, n_cls).
    """
    order = np.argsort(-(main_cnt + cls_cnt), kind="stable")
    gmain = [0] * 8
    gcls = [0] * 8
    gpart = [0] * 8
    ugroup = [0] * N_UNITS
    for ui in order:
        npart = UNIT_NPART[UNITS[ui][0]]
        best, bestv = -1, None
        for g in range(8):
            if gpart[g] + npart > 16:
                continue
            v = gmain[g] + gcls[g]
            if bestv is None or v < bestv:
                best, bestv = g, v
        assert best >= 0, "unit packing overflow"
        ugroup[ui] = best
        gmain[best] += int(main_cnt[ui])
        gcls[best] += int(cls_cnt[ui])
        gpart[best] += npart
    return ugroup, gmain, gcls


def _build_bass(NV, C0, WB, NCLS):
    nc = bacc.Bacc("TRN2", target_bir_lowering=False)
    NI = NV // 16
    MEGA = nc.dram_tensor("mega", [128, WB], mybir.dt.uint8, kind="ExternalInput")
    OUT = nc.dram_tensor("out", [128, 4], mybir.dt.float32, kind="ExternalOutput")

    o_data = 0
    o_tm = 6144
    o_idx = o_tm + 8 * NV
    o_mc = o_idx + 2 * NI

    f32 = mybir.dt.float32
    with tile.TileContext(nc) as tc:
        with tc.tile_pool(name="p", bufs=1) as pool:
            mega = pool.tile([128, WB], mybir.dt.uint8)
            # E1b: dummy gather first, with zero DMA dependencies, so whatever
            # drain/ucode-warm cost precedes an INDIRECT_COPY is paid while
            # the input DMAs stream in
            dg = pool.tile([128, 64], mybir.dt.float32)
            dsrc = pool.tile([128, 64], mybir.dt.float32)
            dgi = pool.tile([128, 4], mybir.dt.uint16)
            nc.vector.memset(dsrc[:], 0)
            nc.vector.memset(dgi[:], 0)
            nc.gpsimd.indirect_copy(
                dg[:], dsrc[:], dgi[:], i_know_ap_gather_is_preferred=True
            )
            # phase A: data + idx (what the gather needs)
            nc.sync.dma_start(mega[:, o_data:6144], MEGA[:, o_data:6144])
            nc.sync.dma_start(mega[:, o_idx:o_mc], MEGA[:, o_idx:o_mc])
            # phase B: targets + cls mask (needed only after the gather)
            nc.sync.dma_start(mega[:, o_tm:o_idx], MEGA[:, o_tm:o_idx])
            nc.sync.dma_start(mega[:, o_mc:WB], MEGA[:, o_mc:WB])

            # hoist both activation-table loads off the critical path: these
            # dummy ops touch every func class we use before the gather runs
            warm = pool.tile([128, 4], f32)
            nc.scalar.activation(warm[:, 0:2], warm[:, 2:4],
                                 mybir.ActivationFunctionType.Ln)
            nc.scalar.activation(warm[:, 0:2], warm[:, 2:4],
                                 mybir.ActivationFunctionType.Exp)
            nc.scalar.activation(warm[:, 0:2], warm[:, 2:4],
                                 mybir.ActivationFunctionType.Abs)
            nc.scalar.activation(warm[:, 0:2], warm[:, 2:4],
                                 mybir.ActivationFunctionType.Square)

            data_v = mega[:, o_data:6144].bitcast(f32)           # [128,1536]
            idx_v = mega[:, o_idx:o_idx + 2 * NI].bitcast(mybir.dt.uint16)
            tm_v = mega[:, o_tm:o_tm + 8 * NV].bitcast(f32)      # [128,2NV]
            mcls_v = mega[:, o_mc:o_mc + NCLS]                   # u8 [128,NCLS]

            g = pool.tile([128, NV], f32)
            nc.gpsimd.indirect_copy(
                g[:], data_v, idx_v, i_know_ap_gather_is_preferred=True
            )

            # dm[p, r, k] = g[p, k] - TM[p, r, k]; TM defaults to the value
            # the gather produces, so non-anchor slots give exactly 0
            gb = g[:, None, :].to_broadcast([128, 2, NV])
            dm = pool.tile([128, 2 * NV], f32)
            nc.vector.tensor_tensor(dm[:].rearrange("p (r k) -> p r k", r=2),
                                    gb,
                                    tm_v.rearrange("p (r k) -> p r k", r=2),
                                    op=mybir.AluOpType.subtract)

            P = pool.tile([128, 4], f32)
            # A = |dm| on the scalar engine (Abs is in every act table);
            # its accum_out gives P[:,0] = sum(|dm|) for free
            A = pool.tile([128, 2 * NV], f32)
            nc.scalar.activation(A[:], dm[:],
                                 mybir.ActivationFunctionType.Abs,
                                 accum_out=P[:, 0:1])
            # m = min(|dm|, 1)
            m = pool.tile([128, 2 * NV], f32)
            nc.vector.tensor_scalar(m[:], A[:], 1.0, None,
                                    mybir.AluOpType.min)
            # P[:,1] = sum(m)
            nc.vector.tensor_reduce(P[:, 1:2], m[:],
                                    axis=mybir.AxisListType.X,
                                    op=mybir.AluOpType.add)
            # P[:,2] = sum(m*m) via ACT Square with accumulate
            sq = pool.tile([128, 2 * NV], f32)
            nc.scalar.activation(sq[:], m[:],
                                 mybir.ActivationFunctionType.Square,
                                 accum_out=P[:, 2:3])

            # classification tail: columns [C0, NV) hold 2*NCLS gathered
            # logits, pair-adjacent; ce = softplus(first - second)
            dc = pool.tile([128, NCLS], f32)
            nc.vector.tensor_tensor(dc[:], g[:, C0:NV:2], g[:, C0 + 1:NV:2],
                                    op=mybir.AluOpType.subtract)
            # ce = softplus(d) = ln(exp(d) + 1); Exp and Ln share one
            # activation table (natural_log_exp_and_others)
            ex = pool.tile([128, NCLS], f32)
            nc.scalar.activation(ex[:], dc[:],
                                 mybir.ActivationFunctionType.Exp)
            ce = pool.tile([128, NCLS], f32)
            nc.scalar.activation(ce[:], ex[:],
                                 mybir.ActivationFunctionType.Ln, bias=1.0)
            cj = pool.tile([128, NCLS], f32)
            nc.vector.tensor_tensor(cj[:], ce[:], mcls_v,
                                    op=mybir.AluOpType.mult)
            nc.vector.tensor_reduce(P[:, 3:4], cj[:],
                                    axis=mybir.AxisListType.X,
                                    op=mybir.AluOpType.add)

            nc.sync.dma_start(OUT[:, :], P[:])
    nc.compile()
    return nc


def kernel(**inputs):
    score = np.asarray(inputs["score"], dtype=np.float32)[0]            # [20,H,W]
    vp = np.asarray(inputs["vertical_pred"], dtype=np.float32)[0]
    side = np.asarray(inputs["side_refinement"], dtype=np.float32)[0]   # [10,H,W]
    pidx = np.asarray(inputs["positive"])
    nidx = np.asarray(inputs["negative"])
    vidx = np.asarray(inputs["vertical_reg_idx"])
    vtgt = np.asarray(inputs["vertical_reg_tgt"], dtype=np.float32)
    sidx = np.asarray(inputs["side_reg_idx"])
    stgt = np.asarray(inputs["side_reg_tgt"], dtype=np.float32)

    score_f = score.reshape(2 * K, HW)
    vp_f = vp.reshape(2 * K, HW)
    side_f = side.reshape(K, HW)

    def fields(idx):
        x = idx[:, 0].astype(np.int64)
        y = idx[:, 1].astype(np.int64)
        a = idx[:, 2].astype(np.int64)
        pos = y * W + x
        return a, pos // PPC, pos % PPC

    va, vcore, vposl = fields(vidx)
    sa, score_, sposl = fields(sidx)
    pa, pcore, pposl = fields(pidx)
    na, ncore, nposl = fields(nidx)

    # --- per (core, unit) entry lists -------------------------------------
    # main entries: vp + sd; cls entries: sc (two idx slots per anchor)
    v_h = vposl // COLS
    v_u = (vposl % COLS).astype(np.int64)
    v_unit = (va * 2 + v_h).astype(np.int64)                 # vp units 0..19
    s_h = sposl // COLS
    s_u = (sposl % COLS).astype(np.int64)
    s_unit = (20 + sa * 2 + s_h).astype(np.int64)            # sd units 20..39
    p_q = pposl // QCOLS
    p_u = (2 * (pposl % QCOLS)).astype(np.int64)
    p_unit = (40 + pa * 4 + p_q).astype(np.int64)            # sc units 40..79
    n_q = nposl // QCOLS
    n_u = (2 * (nposl % QCOLS)).astype(np.int64)
    n_unit = (40 + na * 4 + n_q).astype(np.int64)

    main_core = np.concatenate([vcore, score_])
    main_unit = np.concatenate([v_unit, s_unit])
    main_u = np.concatenate([v_u, s_u])
    main_t0 = np.concatenate([vtgt[:, 0], stgt])
    main_t1 = np.concatenate([vtgt[:, 1], np.zeros_like(stgt)])
    main_isv = np.concatenate(
        [np.ones(len(vidx), np.bool_), np.zeros(len(sidx), np.bool_)])

    cls_core = np.concatenate([pcore, ncore])
    cls_unit = np.concatenate([p_unit, n_unit])
    cls_u = np.concatenate([p_u, n_u])
    cls_ispos = np.concatenate(
        [np.ones(len(pidx), np.bool_), np.zeros(len(nidx), np.bool_)])

    main_cnt = np.zeros((N_CORES, N_UNITS), np.int64)
    np.add.at(main_cnt, (main_core, main_unit), 1)
    cls_cnt = np.zeros((N_CORES, N_UNITS), np.int64)
    np.add.at(cls_cnt, (cls_core, cls_unit), 2)

    # --- pack units into groups per core ----------------------------------
    packs = [_pack_units(main_cnt[c], cls_cnt[c]) for c in range(N_CORES)]
    c0 = max(max(p[1]) for p in packs)
    c0 += c0 % 2
    max_cls = max(max(p[2]) for p in packs)
    NV = c0 + max_cls
    NV = ((NV + 15) // 16) * 16
    NCLS = (NV - c0) // 2
    NI = NV // 16
    WB = 6144 + 8 * NV + 2 * NI + NCLS
    WB = ((WB + 3) // 4) * 4

    key = (NV, c0)
    if key not in _cache:
        _cache[key] = _build_bass(NV, c0, WB, NCLS)
    nc = _cache[key]

    o_tm = 6144
    o_idx = o_tm + 8 * NV
    o_mc = o_idx + 2 * NI

    in_maps = []
    wvec_v = np.zeros((N_CORES, 128), np.float32)
    wvec_o = np.zeros((N_CORES, 128), np.float32)
    for c in range(N_CORES):
        ugroup, gmain, gcls = packs[c]
        # partition layout: group g owns partitions 16g..16g+15, assigned in
        # unit-pack order
        gnext = [16 * g for g in range(8)]
        upart = [0] * N_UNITS
        for ui in range(N_UNITS):
            g = ugroup[ui]
            upart[ui] = gnext[g]
            gnext[g] += UNIT_NPART[UNITS[ui][0]]
            assert gnext[g] <= 16 * g + 16

        data = np.zeros((128, COLS), np.float32)
        base = c * PPC
        for ui, (kind, a, hq) in enumerate(UNITS):
            p0 = upart[ui]
            if kind == "vp":
                sl = slice(base + hq * COLS, base + (hq + 1) * COLS)
                data[p0] = vp_f[2 * a, sl]
                data[p0 + 1] = vp_f[2 * a + 1, sl]
                wvec_v[c, p0] = wvec_v[c, p0 + 1] = 1.0 / (2.0 * NV_REG)
            elif kind == "sd":
                sl = slice(base + hq * COLS, base + (hq + 1) * COLS)
                data[p0] = side_f[a, sl]
                wvec_o[c, p0] = 1.0 / NO_REG
            else:  # sc, pair-interleaved quarter
                sl = slice(base + hq * QCOLS, base + (hq + 1) * QCOLS)
                data[p0, 0::2] = score_f[2 * a, sl]
                data[p0, 1::2] = score_f[2 * a + 1, sl]

        idxs = np.zeros((128, NI), np.uint16)
        ucol = np.zeros((8, NV), np.int64)    # per-group gathered column
        mcls = np.zeros((128, NCLS), np.uint8)

        gq_main = [0] * 8   # next main col per group
        gq_cls = [0] * 8    # next cls PAIR slot per group

        def put_idx(g, col, val):
            idxs[16 * g + col % 16, col // 16] = val
            ucol[g, col] = val

        # main entries: remember (partition, r, col, target) to overwrite
        ov_p, ov_r, ov_c, ov_t = [], [], [], []
        msel = main_core == c
        for u, ui, t0, t1, isv in zip(main_u[msel], main_unit[msel],
                                      main_t0[msel], main_t1[msel],
                                      main_isv[msel]):
            g = ugroup[ui]
            col = gq_main[g]
            gq_main[g] += 1
            put_idx(g, col, u)
            p0 = upart[ui]
            ov_p.append(p0); ov_r.append(0); ov_c.append(col); ov_t.append(t0)
            if isv:
                ov_p.append(p0 + 1); ov_r.append(1); ov_c.append(col)
                ov_t.append(t1)

        csel = cls_core == c
        for u, ui, ispos in zip(cls_u[csel], cls_unit[csel],
                                cls_ispos[csel]):
            g = ugroup[ui]
            i = gq_cls[g]
            gq_cls[g] += 1
            colf = c0 + 2 * i
            # pos: (l0, l1); neg: (l1, l0) -> ce = softplus(first - second)
            if ispos:
                put_idx(g, colf, u)
                put_idx(g, colf + 1, u + 1)
            else:
                put_idx(g, colf, u + 1)
                put_idx(g, colf + 1, u)
            mcls[upart[ui], i] = 1

        # TM defaults to exactly what the gather will produce (so junk
        # slots subtract to 0), then anchor slots get their real targets
        tm = np.empty((128, 2, NV), np.float32)
        for g in range(8):
            sl = data[16 * g:16 * g + 16][:, ucol[g]]   # [16, NV]
            tm[16 * g:16 * g + 16, 0, :] = sl
            tm[16 * g:16 * g + 16, 1, :] = sl
        if ov_p:
            tm[np.array(ov_p), np.array(ov_r), np.array(ov_c)] = \
                np.array(ov_t, np.float32)

        mega = np.zeros((128, WB), np.uint8)
        mega[:, 0:6144] = data.view(np.uint8).reshape(128, 6144)
        mega[:, o_tm:o_tm + 8 * NV] = tm.view(np.uint8).reshape(128, 8 * NV)
        mega[:, o_idx:o_idx + 2 * NI] = idxs.view(np.uint8).reshape(128, 2 * NI)
        mega[:, o_mc:o_mc + NCLS] = mcls
        in_maps.append({"mega": mega})

    res = bass_utils.run_bass_kernel_spmd(
        nc, in_maps, core_ids=list(range(N_CORES)))

    v_loss = np.float32(0.0)
    o_loss = np.float32(0.0)
    cls_sum = np.float32(0.0)
    for c in range(N_CORES):
        P = res.results[c]["out"]      # [128, 4]
        S = 0.5 * P[:, 2] + P[:, 0] - P[:, 1]
        v_loss += np.float32(np.dot(S, wvec_v[c]))
        o_loss += np.float32(np.dot(S, wvec_o[c]))
        cls_sum += np.float32(P[:, 3].sum())
    cls_loss = np.float32(cls_sum / NS)
    loss = np.float32(cls_loss + v_loss + o_loss)
    return (np.float32(loss), np.float32(cls_loss), np.float32(v_loss),
            np.float32(o_loss))



# revision 10
# speedup vs baseline: 1.9146x; 1.9146x over previous
"""CTPN loss kernel for 8 Trainium2 NeuronCores — dense-slab design.

Strategy (data parallel over positions, no GPSIMD custom ops):
  * The H*W=24576 spatial positions are split into 8 contiguous slices of
    3072; core c holds all 50 map channels for its slice in bf16.
  * Smooth-L1 terms are evaluated DENSELY over the vertical/side maps: the
    host builds a target grid TM that defaults to the map values themselves,
    so un-referenced cells contribute exactly 0, and writes the regression
    target at each referenced cell (duplicate references spill into a small
    host-filled overflow region).  Using
        sl1(d) = 0.5*d^2 - 0.5*(d - clamp(d,-1,1))^2
    the whole reduction is two ACT Square+accumulate passes plus three DVE
    passes (subtract, fused clamp, subtract) — no gather.
  * Classification CE is likewise dense: the score maps are laid out as
    (first, second) logit pairs per anchor-position — the host pre-swaps the
    pair order for negative-labelled cells — and a per-cell bf16 count grid
    weights softplus(first - second); CE = sum(W * softplus(D)).
  * Per-partition partial sums return to the host, which applies the
    divisors (1/(2*Nv), 1/No, 1/Ns) and sums across cores (the all-reduce).
  * All activations (Square, Softplus) live in one ACT table, loaded once
    off the critical path while the input DMAs stream.
"""

import sys

sys.path.insert(0, "/opt/trn_rl_repo")

import numpy as np

import concourse.bacc as bacc
import concourse.tile as tile
from concourse import mybir
from concourse import bass_utils

# ---------------- problem constants (hardcoded per contract) ----------------
H, W, K = 128, 192, 10
HW = H * W                      # 24576
N_CORES = 8
PPC = HW // N_CORES             # 3072 positions per core
NS = 128.0
NV_REG = 20000
NO_REG = 5000

F = 768                         # slab free dim (elements per partition)
VP_PARTS = 80                   # 20 ch * 3072 / 768
SD_PARTS = 40                   # 10 ch * 3072 / 768
OV0 = VP_PARTS + SD_PARTS       # overflow partitions 120..127
NPAIR = 240                     # score pairs per partition (10*3072/128)
OVP = 16                        # overflow pair slots per partition
NPW = NPAIR + OVP               # W grid width = 256
SCW = 2 * NPW                   # score cols = 512

o_data = 0
o_tm = 2 * F                    # 1536
o_sc = 4 * F                    # 3072
o_w = o_sc + 2 * SCW            # 4096
WB = o_w + 2 * NPW              # 4608 bytes per partition

_cache = {}


def _bf16(x):
    """Round f32 -> bf16 (RNE), return uint16 bit patterns."""
    u = np.asarray(x, np.float32).view(np.uint32)
    r = (u + 0x7FFF + ((u >> 16) & 1)) >> 16
    return r.astype(np.uint16)


def _build_bass():
    nc = bacc.Bacc("TRN2", target_bir_lowering=False)
    MEGA = nc.dram_tensor("mega", [128, WB], mybir.dt.uint8, kind="ExternalInput")
    OUT = nc.dram_tensor("out", [128, 4], mybir.dt.float32, kind="ExternalOutput")

    f32 = mybir.dt.float32
    bf16 = mybir.dt.bfloat16
    AF = mybir.ActivationFunctionType
    with tile.TileContext(nc) as tc:
        with tc.tile_pool(name="p", bufs=1) as pool:
            # explicit single ACT-table load (natural_log_exp_and_others,
            # id 6: has Square+Exp+Ln+Abs) — placed first on the scalar
            # queue so it overlaps the input DMAs, and it suppresses all
            # auto-inserted per-activation table loads
            nc.scalar.add_instruction(mybir.InstLoadActFuncSet(
                name=nc.get_next_instruction_name(), ins=[], outs=[],
                act_func_set_id=6))

            mega = pool.tile([128, WB], mybir.dt.uint8)
            # inputs: slab (data+tm) on the sync queue, cls tensors second
            nc.sync.dma_start(mega[:, o_data:o_sc], MEGA[:, o_data:o_sc])
            nc.sync.dma_start(mega[:, o_sc:WB], MEGA[:, o_sc:WB])

            data_v = mega[:, o_data:o_tm].bitcast(bf16)   # [128, 768]
            tm_v = mega[:, o_tm:o_sc].bitcast(bf16)       # [128, 768]
            sc_v = mega[:, o_sc:o_w].bitcast(bf16)        # [128, 512]
            w_v = mega[:, o_w:WB].bitcast(bf16)           # [128, 256]

            P = pool.tile([128, 4], f32)
            nc.vector.memset(P[:], 0.0)

            # d = v - t  (0 on every un-referenced cell)
            dm = pool.tile([128, F], bf16)
            nc.vector.tensor_tensor(dm[:], data_v, tm_v,
                                    op=mybir.AluOpType.subtract)
            # P0 = sum d^2
            sq = pool.tile([128, F], bf16)
            nc.scalar.activation(sq[:], dm[:], AF.Square,
                                 accum_out=P[:, 0:1])
            # c = clamp(d, -1, 1) in one fused tensor_scalar
            c = pool.tile([128, F], bf16)
            nc.vector.tensor_scalar(c[:], dm[:], 1.0, -1.0,
                                    mybir.AluOpType.min, mybir.AluOpType.max)
            # e = d - c = sign(d)*relu(|d|-1);  P1 = sum e^2
            e = pool.tile([128, F], bf16)
            nc.vector.tensor_tensor(e[:], dm[:], c[:],
                                    op=mybir.AluOpType.subtract)
            nc.scalar.activation(sq[:], e[:], AF.Square,
                                 accum_out=P[:, 1:2])

            # classification: D = first - second, CE = sum W * softplus(D)
            # softplus(D) = ln(exp(D) + 1); Exp and Ln live in table 6 too.
            # This tail is tiny, so it runs in f32 for accuracy.
            D = pool.tile([128, NPW], f32)
            nc.vector.tensor_tensor(D[:], sc_v[:, 0::2], sc_v[:, 1::2],
                                    op=mybir.AluOpType.subtract)
            ex = pool.tile([128, NPW], f32)
            nc.scalar.activation(ex[:], D[:], AF.Exp)
            ce = pool.tile([128, NPW], f32)
            nc.scalar.activation(ce[:], ex[:], AF.Ln, bias=1.0)
            wsp = pool.tile([128, NPW], f32)
            nc.vector.tensor_tensor(wsp[:], ce[:], w_v,
                                    op=mybir.AluOpType.mult)
            nc.vector.tensor_reduce(P[:, 2:3], wsp[:],
                                    axis=mybir.AxisListType.X,
                                    op=mybir.AluOpType.add)

            nc.sync.dma_start(OUT[:, :], P[:])
    nc.compile()
    return nc


def kernel(**inputs):
    score = np.asarray(inputs["score"], dtype=np.float32)[0].reshape(2 * K, HW)
    vp = np.asarray(inputs["vertical_pred"], dtype=np.float32)[0].reshape(2 * K, HW)
    side = np.asarray(inputs["side_refinement"], dtype=np.float32)[0].reshape(K, HW)
    pidx = np.asarray(inputs["positive"])
    nidx = np.asarray(inputs["negative"])
    vidx = np.asarray(inputs["vertical_reg_idx"])
    vtgt = np.asarray(inputs["vertical_reg_tgt"], dtype=np.float32)
    sidx = np.asarray(inputs["side_reg_idx"])
    stgt = np.asarray(inputs["side_reg_tgt"], dtype=np.float32)

    vp_b = _bf16(vp)        # [20, HW] uint16
    side_b = _bf16(side)    # [10, HW]
    score_b = _bf16(score)  # [20, HW]

    def fields(idx):
        x = idx[:, 0].astype(np.int64)
        y = idx[:, 1].astype(np.int64)
        a = idx[:, 2].astype(np.int64)
        pos = y * W + x
        return a, pos // PPC, pos % PPC

    va, vcore, vposl = fields(vidx)
    sa, score_, sposl = fields(sidx)
    pa, pcore, pposl = fields(pidx)
    na, ncore, nposl = fields(nidx)

    # --- sl1 cell refs: vp entries contribute 2 cells (ch 2a, 2a+1) --------
    # cell id within a core = part*F + col; vp block parts [0,80), sd [80,120)
    v_j0 = (2 * va) * PPC + vposl          # flat (ch, posl), ch-major
    v_j1 = (2 * va + 1) * PPC + vposl
    s_j = sa * PPC + sposl

    ref_core = np.concatenate([vcore, vcore, score_])
    ref_cell = np.concatenate([v_j0, v_j1, s_j + VP_PARTS * F])
    ref_tgt = np.concatenate([vtgt[:, 0], vtgt[:, 1], stgt]).astype(np.float32)
    ref_isv = np.concatenate([np.ones(2 * len(vidx), np.bool_),
                              np.zeros(len(sidx), np.bool_)])

    # --- cls pair-cell refs ------------------------------------------------
    cls_core = np.concatenate([pcore, ncore])
    cls_q = np.concatenate([pa * PPC + pposl, na * PPC + nposl])
    cls_isneg = np.concatenate([np.zeros(len(pidx), np.bool_),
                                np.ones(len(nidx), np.bool_)])

    if "nc" not in _cache:
        _cache["nc"] = _build_bass()
    nc = _cache["nc"]

    in_maps = []
    wvec_v = np.zeros((N_CORES, 128), np.float64)
    wvec_o = np.zeros((N_CORES, 128), np.float64)
    for cidx in range(N_CORES):
        sl = slice(cidx * PPC, (cidx + 1) * PPC)
        # slab data: [128, 768] uint16; vp rows then sd rows
        slab = np.zeros((128, F), np.uint16)
        slab[:VP_PARTS] = vp_b[:, sl].reshape(VP_PARTS, F)
        slab[VP_PARTS:OV0] = side_b[:, sl].reshape(SD_PARTS, F)
        tm = slab.copy()

        # weights for the regular blocks
        wvec_v[cidx, :VP_PARTS] = 1.0 / (2.0 * NV_REG)
        wvec_o[cidx, VP_PARTS:OV0] = 1.0 / NO_REG

        # write targets; duplicates go to the overflow partitions
        msel = ref_core == cidx
        cells = ref_cell[msel]
        tgts = _bf16(ref_tgt[msel])
        isv = ref_isv[msel]
        _, first = np.unique(cells, return_index=True)
        tm.reshape(-1)[cells[first]] = tgts[first]
        extra = np.ones(len(cells), np.bool_)
        extra[first] = False
        ev_cells, ev_t = cells[extra & isv], tgts[extra & isv]
        eo_cells, eo_t = cells[extra & ~isv], tgts[extra & ~isv]
        # vp extras fill overflow rows from 120 up, sd extras from 127 down
        nv_rows = (len(ev_cells) + F - 1) // F
        no_rows = (len(eo_cells) + F - 1) // F
        assert nv_rows + no_rows <= 128 - OV0, "overflow region full"
        flat_slab = slab.reshape(-1)
        if len(ev_cells):
            base = OV0 * F
            idxs = base + np.arange(len(ev_cells))
            flat_slab[idxs] = flat_slab[ev_cells]
            tm.reshape(-1)[idxs] = ev_t
            wvec_v[cidx, OV0:OV0 + nv_rows] = 1.0 / (2.0 * NV_REG)
        if len(eo_cells):
            base = 128 * F - len(eo_cells)
            idxs = base + np.arange(len(eo_cells))
            flat_slab[idxs] = flat_slab[eo_cells]
            tm.reshape(-1)[idxs] = eo_t
            wvec_o[cidx, 128 - no_rows:128] = 1.0 / NO_REG

        # --- score pairs + weight grid ------------------------------------
        # pair slot q = a*3072 + posl -> (part, slot) = (q//240, q%240)
        pair = np.empty((K * PPC, 2), np.uint16)
        pair[:, 0] = score_b[0::2, sl].reshape(-1)
        pair[:, 1] = score_b[1::2, sl].reshape(-1)
        cp = np.zeros(K * PPC, np.int64)
        cn = np.zeros(K * PPC, np.int64)
        csel = cls_core == cidx
        q_here = cls_q[csel]
        neg_here = cls_isneg[csel]
        np.add.at(cp, q_here[~neg_here], 1)
        np.add.at(cn, q_here[neg_here], 1)
        both = (cp > 0) & (cn > 0)
        swap = (cn > 0) & (cp == 0)
        pair[swap] = pair[swap][:, ::-1]
        wgrid = np.where(cp > 0, cp, cn).astype(np.float32)
        wgrid[~((cp > 0) | (cn > 0))] = 0.0

        pair = pair.reshape(128, NPAIR, 2)
        wgrid = wgrid.reshape(128, NPAIR)
        # overflow pair slots for cells with both pos and neg refs
        ov_pair = np.zeros((128, OVP, 2), np.uint16)
        ov_w = np.zeros((128, OVP), np.float32)
        bq = np.nonzero(both)[0]
        assert len(bq) <= 128 * OVP, "cls overflow full"
        for i, q in enumerate(bq):
            p_, s_ = i // OVP, i % OVP
            ov_pair[p_, s_, 0] = pair[q // NPAIR, q % NPAIR, 1]
            ov_pair[p_, s_, 1] = pair[q // NPAIR, q % NPAIR, 0]
            ov_w[p_, s_] = cn[q]

        sc_full = np.concatenate([pair, ov_pair], axis=1)   # [128, 256, 2]
        w_full = np.concatenate([wgrid, ov_w], axis=1)      # [128, 256]

        mega = np.empty((128, WB), np.uint8)
        mega[:, o_data:o_tm] = slab.view(np.uint8).reshape(128, 2 * F)
        mega[:, o_tm:o_sc] = tm.view(np.uint8).reshape(128, 2 * F)
        mega[:, o_sc:o_w] = sc_full.view(np.uint8).reshape(128, 2 * SCW)
        mega[:, o_w:WB] = _bf16(w_full).view(np.uint8).reshape(128, 2 * NPW)
        in_maps.append({"mega": mega})

    res = bass_utils.run_bass_kernel_spmd(
        nc, in_maps, core_ids=list(range(N_CORES)))

    v_loss = 0.0
    o_loss = 0.0
    cls_sum = 0.0
    for cidx in range(N_CORES):
        P = res.results[cidx]["out"].astype(np.float64)   # [128, 4]
        S = 0.5 * (P[:, 0] - P[:, 1])
        v_loss += float(np.dot(S, wvec_v[cidx]))
        o_loss += float(np.dot(S, wvec_o[cidx]))
        cls_sum += float(P[:, 2].sum())
    cls_loss = np.float32(cls_sum / NS)
    loss = np.float32(cls_loss + v_loss + o_loss)
    return (np.float32(loss), np.float32(cls_loss), np.float32(v_loss),
            np.float32(o_loss))
